# revision 1
# baseline (speedup 1.0000x reference)
"""ConvDualAttention Trainium2 kernel (Bass/Tile), 8-core data-parallel.

Contract: kernel(**inputs) takes the FULL unsharded inputs, shards batch b
across the 8 NeuronCores (one batch per core), and returns the full
(8, 128, 4096) float32 output.

Math (per batch b, per head h, D=128, X=4096):
  y_p   = dwconv3(x) + t_p/s_p           (p in q,k,v; BN folded so that
                                          W_eff_p @ y_p == pw_p @ BN(conv))
  k     = W_eff_k @ y_k ; sk = softmax(k over d)
  kat   = SCALE * q^T @ sk               (SCALE folded into W_q)
  gout  = GW @ q + gb ; sig = sigmoid(gout)
  out_h = v @ kat + sig^T * v
  out   = out_w @ merge(out_h) + out_b

Kernel factorizations (validated against the jax reference):
  * q is never materialized:  kat_h = wtq_h^T @ R_h + qb_h (x) sigma_h
    with R_h = y_qT^T @ sk'_h (contraction over x), sigma_h = ones^T @ sk'_h,
    y_qT the transposed conv output (produced by bf16 diagonal matmuls),
    qb_h = wtq_h^T @ t'_q the bias part.
  * v@kat through the output projection collapses to W3 @ y_v with
    W3 = sum_h outw_h @ (Wv_h^T @ kat_h)^T, computed on-chip from the
    tiny per-head kat matrices.
"""
import numpy as np
import ml_dtypes

import concourse.bass as bass
import concourse.tile as tile
from concourse import bacc, mybir
from concourse.bass_utils import run_bass_kernel_spmd

F32 = mybir.dt.float32
F32R = mybir.dt.float32r
BF16 = mybir.dt.bfloat16
AF = mybir.ActivationFunctionType
ALU = mybir.AluOpType

B = 8
DIM = 128
HEADS = 8
INNER = DIM * HEADS
X = 4096
EPS = 1e-5
SCALE = DIM ** -0.5
NT = X // 128          # 32 x-tiles of 128
NCH = X // 512         # 8 chunks of 512
GROUPS = 2
GH = HEADS // GROUPS   # 4 heads per group

_NC = None
TRACE = False
LAST_EXEC_NS = None


def _bf(a):
    return np.ascontiguousarray(np.asarray(a, np.float32).astype(ml_dtypes.bfloat16))


def _prep(inputs):
    """Host-side weight folding. Returns dict of DRAM input arrays."""
    f = lambda k: np.asarray(inputs[k], np.float32)
    wt = {}
    tprime = {}
    diag_cols = []
    dwq_cols = []
    for p in ("q", "k", "v"):
        s = f(p + "_g") / np.sqrt(f(p + "_v") + EPS)        # (128,)
        t = f(p + "_b") - f(p + "_m") * s
        tprime[p] = t / s
        w_eff = f(p + "_pw") * s[None, :]                    # (1024, 128)
        wt[p] = np.ascontiguousarray(w_eff.T)                # (128, 1024)
        dw = f(p + "_dw")[:, 0, :]                           # (128, 3)
        for j in range(3):
            d = np.diag(dw[:, j]).astype(np.float32)
            diag_cols.append(d)
            if p == "q":
                dwq_cols.append(d)
    s_gt = f("gt_g") / np.sqrt(f("gt_v") + EPS)
    t_gt = f("gt_b") - f("gt_m") * s_gt
    gw = f("gt_pw") * (f("gt_dw")[:, 0, 0] * s_gt)[None, :]  # (128, 128)
    gb = f("gt_pw") @ t_gt                                   # (128,)
    w_eff_q = wt["q"].T                                      # (1024, 128)
    gqt = np.concatenate(
        [(gw @ w_eff_q[h * 128:(h + 1) * 128, :]).T for h in range(HEADS)], axis=1
    )                                                        # (128 i, 1024 h*o)
    out_w = f("out_w")                                       # (128, 1024)
    outwt = np.concatenate(
        [np.ascontiguousarray(out_w[:, h * 128:(h + 1) * 128].T) for h in range(HEADS)],
        axis=1,
    )                                                        # (128 d, 1024 h*o)
    wvdm = np.concatenate(
        [wt["v"].T[h * 128:(h + 1) * 128, :] for h in range(HEADS)], axis=1
    )                                                        # (128 d, 1024 h*i)
    diag = np.concatenate(diag_cols, axis=1)                 # (128, 1152)
    wtq_s = wt["q"] * SCALE                                  # (128 i, 1024 d)
    qb = (wtq_s.T @ tprime["q"]).reshape(1, INNER)           # (1, 1024)
    biasp = np.stack(
        [tprime["q"], tprime["k"], tprime["v"], gb, f("out_b")], axis=1
    )                                                        # (128, 5)
    return {
        "wtk": np.ascontiguousarray(wt["k"]),
        "wtv": np.ascontiguousarray(wt["v"]),
        "gqt": np.ascontiguousarray(gqt),
        "outwt": np.ascontiguousarray(outwt),
        "wvdm": np.ascontiguousarray(wvdm),
        "diag": np.ascontiguousarray(diag),
        "biasp": np.ascontiguousarray(biasp),
        "wtqr": np.ascontiguousarray(wtq_s),
        "qb": np.ascontiguousarray(qb.astype(np.float32)),
        "diagq": _bf(np.concatenate(dwq_cols, axis=1)),      # (128, 384)
    }


def _build():
    nc = bacc.Bacc("TRN2", target_bir_lowering=False, debug=False, num_devices=B)
    x_d = nc.dram_tensor("x", [128, X + 2], F32R, kind="ExternalInput").ap()
    xb_d = nc.dram_tensor("xb", [128, X + 2], BF16, kind="ExternalInput").ap()
    wtk_d = nc.dram_tensor("wtk", [128, INNER], F32R, kind="ExternalInput").ap()
    wtv_d = nc.dram_tensor("wtv", [128, INNER], F32R, kind="ExternalInput").ap()
    gqt_d = nc.dram_tensor("gqt", [128, INNER], F32R, kind="ExternalInput").ap()
    outwt_d = nc.dram_tensor("outwt", [128, INNER], F32R, kind="ExternalInput").ap()
    wvdm_d = nc.dram_tensor("wvdm", [128, INNER], F32R, kind="ExternalInput").ap()
    diag_d = nc.dram_tensor("diag", [128, 9 * 128], F32R, kind="ExternalInput").ap()
    biasp_d = nc.dram_tensor("biasp", [128, 5], F32, kind="ExternalInput").ap()
    wtqr_d = nc.dram_tensor("wtqr", [128, INNER], F32R, kind="ExternalInput").ap()
    qb_d = nc.dram_tensor("qb", [1, INNER], F32R, kind="ExternalInput").ap()
    diagq_d = nc.dram_tensor("diagq", [128, 3 * 128], BF16, kind="ExternalInput").ap()
    out_d = nc.dram_tensor("out", [128, X], F32, kind="ExternalOutput").ap()

    with tile.TileContext(nc) as tc:
        with (
            tc.tile_pool(name="const", bufs=1) as cp,
        ):
            wtk = cp.tile([128, INNER], F32R)
            wtv = cp.tile([128, INNER], F32R)
            gqt = cp.tile([128, INNER], F32R)
            outwt = cp.tile([128, INNER], F32R)
            wvdm = cp.tile([128, INNER], F32R)
            biasp = cp.tile([128, 5], F32)
            wtqr = cp.tile([128, INNER], F32R)
            qbsb = cp.tile([1, INNER], F32R)
            yq = cp.tile([128, X], F32R, tag="yq")
            yk = cp.tile([128, X], F32R, tag="yk")
            yv = cp.tile([128, X], F32R, tag="yv")
            yqt = cp.tile([128, X], BF16, tag="yqt")
            onescol = cp.tile([128, 1], BF16)
            onesb = cp.tile([128, 128], BF16)
            zt = cp.tile([128, 2 * NT * GH], F32, tag="zt")
            zi = cp.tile([128, 2 * NT * GH], F32, tag="zi")
            sgsb = cp.tile([1, INNER], F32R, tag="sgsb")
            w3t_sb = cp.tile([128, 128], F32R, tag="w3t")

            nc.vector.memset(onescol, 1.0)
            nc.vector.memset(onesb, 1.0)

            ys = {"q": yq, "k": yk, "v": yv}

            # ---- y-stage: depthwise conv via 3 shifted diagonal matmuls ----
            with (
                tc.tile_pool(name="xp", bufs=1) as xp,
                tc.tile_pool(name="yps", bufs=2, space="PSUM") as yps,
            ):
                xpad = xp.tile([128, X + 2], F32R)
                xpb = xp.tile([128, X + 2], BF16)
                diag = xp.tile([128, 9 * 128], F32R)
                diagqb = xp.tile([128, 3 * 128], BF16)
                nc.sync.dma_start(out=xpad, in_=x_d)
                nc.sync.dma_start(out=diag, in_=diag_d)
                nc.sync.dma_start(out=biasp, in_=biasp_d)
                nc.sync.dma_start(out=xpb, in_=xb_d)
                nc.sync.dma_start(out=diagqb, in_=diagq_d)
                for sb_t, dr in ((wtk, wtk_d), (wtqr, wtqr_d), (qbsb, qb_d),
                                 (wvdm, wvdm_d), (outwt, outwt_d),
                                 (gqt, gqt_d), (wtv, wtv_d)):
                    nc.sync.dma_start(out=sb_t, in_=dr)
                for pi, p in enumerate(("q", "k", "v")):
                    for c in range(NCH):
                        pt = yps.tile([128, 512], F32, tag="yps")
                        for j in range(3):
                            dsl = diag[:, (pi * 3 + j) * 128:(pi * 3 + j + 1) * 128]
                            nc.tensor.matmul(
                                pt, dsl,
                                xpad[:, c * 512 + j:c * 512 + j + 512],
                                start=(j == 0), stop=(j == 2),
                            )
                        nc.scalar.activation(
                            ys[p][:, c * 512:(c + 1) * 512], pt,
                            AF.Identity, bias=biasp[:, pi:pi + 1],
                        )
                # y_qT: transposed conv output for the q path (bf16, no bias)
                for t in range(NT):
                    qt = yps.tile([128, 128], F32, tag="yqt")
                    for j in range(3):
                        nc.tensor.matmul(
                            qt,
                            xpb[:, t * 128 + j:t * 128 + j + 128],
                            diagqb[:, j * 128:(j + 1) * 128],
                            start=(j == 0), stop=(j == 2),
                        )
                    nc.scalar.copy(yqt[:, t * 128:(t + 1) * 128], qt)

            # ---- phase A: K softmax, R/sigma, kat, M2T, W3T ----
            with (
                tc.tile_pool(name="gp2", bufs=2) as gp2,
                tc.tile_pool(name="small", bufs=2) as sp,
                tc.tile_pool(name="kqps", bufs=3, space="PSUM") as kqps,
                tc.tile_pool(name="rps", bufs=1, space="PSUM") as rps,
                tc.tile_pool(name="sgps", bufs=1, space="PSUM") as sgps,
                tc.tile_pool(name="katps", bufs=1, space="PSUM") as katps,
                tc.tile_pool(name="m2ps", bufs=1, space="PSUM") as m2ps,
                tc.tile_pool(name="w3ps", bufs=1, space="PSUM") as w3ps,
            ):
                w3t_ps = w3ps.tile([128, 128], F32)
                sks = []
                # K production + per-head exp evac with fused Z accumulation
                for g in range(GROUPS):
                    osl = slice(g * 512, (g + 1) * 512)
                    sksb = gp2.tile([128, NT * 512], BF16, tag="sksb")
                    sks.append(sksb)
                    for t in range(NT):
                        kps = kqps.tile([128, 512], F32, tag="kq")
                        nc.tensor.matmul(
                            kps, yk[:, t * 128:(t + 1) * 128], wtk[:, osl],
                            start=True, stop=True,
                        )
                        nc.scalar.activation(
                            sksb[:, t * 512:(t + 1) * 512], kps, AF.Exp,
                        )
                        if t % 4 == 3:
                            t0 = t - 3
                            inr = sksb[:, t0 * 512:(t + 1) * 512].rearrange(
                                "p (t h d) -> p t h d", t=4, h=GH
                            )
                            zb4 = g * NT * GH + t0 * GH
                            nc.vector.tensor_reduce(
                                zt[:, zb4:zb4 + 4 * GH], inr,
                                mybir.AxisListType.X, ALU.add,
                            )
                        if t % 8 == 7:
                            zb8 = g * NT * GH + (t - 7) * GH
                            nc.vector.reciprocal(
                                zi[:, zb8:zb8 + 8 * GH], zt[:, zb8:zb8 + 8 * GH]
                            )
                for g in range(GROUPS):
                    sksb = sks[g]
                    # normalize sk in place (per x-tile, per head)
                    for t in range(NT):
                        for hh in range(GH):
                            col = g * NT * GH + t * GH + hh
                            sl = sksb[:, t * 512 + hh * 128:t * 512 + (hh + 1) * 128]
                            if t % 8 == 0:
                                nc.scalar.activation(
                                    sl, sl, AF.Copy, scale=zi[:, col:col + 1]
                                )
                            else:
                                nc.vector.tensor_scalar(
                                    sl, sl, zi[:, col:col + 1], None, ALU.mult
                                )

                    # R (4 heads wide) and sigma via PE
                    sg_ps = sgps.tile([1, 512], F32, tag="sg")
                    r_ps = rps.tile([128, 512], F32, tag="r")
                    for t in range(NT):
                        nc.tensor.matmul(
                            r_ps, yqt[:, t * 128:(t + 1) * 128],
                            sksb[:, t * 512:(t + 1) * 512],
                            start=(t == 0), stop=(t == NT - 1),
                            skip_group_check=True,
                        )
                        nc.tensor.matmul(
                            sg_ps, onescol, sksb[:, t * 512:(t + 1) * 512],
                            start=(t == 0), stop=(t == NT - 1),
                            skip_group_check=True,
                        )
                    r_sb = sp.tile([128, 512], F32R, tag="rsb")
                    nc.vector.tensor_copy(r_sb, r_ps)
                    nc.vector.tensor_copy(sgsb[0:1, g * 512:(g + 1) * 512], sg_ps)
                    # kat -> M2T -> W3T per head
                    for hh in range(GH):
                        h = g * GH + hh
                        kat_ps = katps.tile([128, 128], F32, tag="katp")
                        nc.tensor.matmul(
                            kat_ps, wtqr[:, h * 128:(h + 1) * 128],
                            r_sb[:, hh * 128:(hh + 1) * 128],
                            start=True, stop=False, skip_group_check=True,
                        )
                        nc.tensor.matmul(
                            kat_ps, qbsb[0:1, h * 128:(h + 1) * 128],
                            sgsb[0:1, h * 128:(h + 1) * 128],
                            start=False, stop=True, skip_group_check=True,
                        )
                        kat_sb = sp.tile([128, 128], F32R, tag="katsb")
                        nc.vector.tensor_copy(kat_sb, kat_ps)
                        m2_ps = m2ps.tile([128, 128], F32, tag="m2")
                        nc.tensor.matmul(
                            m2_ps, kat_sb,
                            wvdm[:, h * 128:(h + 1) * 128],
                            start=True, stop=True, skip_group_check=True,
                        )
                        m2_sb = sp.tile([128, 128], F32R, tag="m2sb")
                        nc.vector.tensor_copy(m2_sb, m2_ps)
                        nc.tensor.matmul(
                            w3t_ps, m2_sb,
                            outwt[:, h * 128:(h + 1) * 128],
                            start=(h == 0), stop=(h == HEADS - 1),
                            skip_group_check=True,
                        )
                nc.vector.tensor_copy(w3t_sb, w3t_ps)

            # ---- phase B: gate + final projection per 512-chunk ----
            with (
                tc.tile_pool(name="goutps", bufs=2, space="PSUM") as goutps,
                tc.tile_pool(name="vps", bufs=2, space="PSUM") as vps,
                tc.tile_pool(name="finps", bufs=2, space="PSUM") as finps,
                tc.tile_pool(name="bpool", bufs=2) as bp,
            ):
                for c in range(NCH):
                    csl = slice(c * 512, (c + 1) * 512)
                    fin_ps = finps.tile([128, 512], F32, tag="fin")
                    nc.tensor.matmul(
                        fin_ps, w3t_sb, yv[:, csl],
                        start=True, stop=False, skip_group_check=True,
                    )
                    for hp in range(HEADS // 2):
                        v_ps = vps.tile([128, 1024], F32, tag="vp")
                        sig = bp.tile([128, 1024], F32, tag="sig")
                        for d in range(2):
                            h = hp * 2 + d
                            g_ps = goutps.tile([128, 512], F32, tag="gout")
                            nc.tensor.matmul(
                                g_ps, gqt[:, h * 128:(h + 1) * 128],
                                yq[:, csl], start=True, stop=True,
                            )
                            nc.scalar.activation(
                                sig[:, d * 512:(d + 1) * 512], g_ps,
                                AF.Sigmoid, bias=biasp[:, 3:4],
                            )
                            nc.tensor.matmul(
                                v_ps[:, d * 512:(d + 1) * 512],
                                wtv[:, h * 128:(h + 1) * 128],
                                yv[:, csl], start=True, stop=True,
                            )
                        gate = bp.tile([128, 1024], F32R, tag="gate")
                        nc.vector.tensor_tensor(gate, v_ps, sig, ALU.mult)
                        for d in range(2):
                            h = hp * 2 + d
                            nc.tensor.matmul(
                                fin_ps, outwt[:, h * 128:(h + 1) * 128],
                                gate[:, d * 512:(d + 1) * 512],
                                start=False, stop=(h == HEADS - 1),
                                skip_group_check=True,
                            )
                    fin_sb = bp.tile([128, 512], F32, tag="finsb")
                    nc.scalar.activation(
                        fin_sb, fin_ps, AF.Identity, bias=biasp[:, 4:5]
                    )
                    nc.sync.dma_start(out=out_d[:, csl], in_=fin_sb)

    nc.compile()
    return nc


def kernel(**inputs):
    global _NC, LAST_EXEC_NS
    host = _prep(inputs)
    if _NC is None:
        _NC = _build()
    x = np.asarray(inputs["x"], np.float32)
    in_maps = []
    for b in range(B):
        xp = np.pad(x[b], ((0, 0), (1, 1)))
        m = {"x": np.ascontiguousarray(xp), "xb": _bf(xp)}
        m.update(host)
        in_maps.append(m)
    res = run_bass_kernel_spmd(
        _NC, in_maps, core_ids=list(range(B)), trace=TRACE
    )
    LAST_EXEC_NS = res.exec_time_ns
    return np.stack([r["out"] for r in res.results]).astype(np.float32)



# revision 5
# speedup vs baseline: 1.2866x; 1.2866x over previous
"""ConvDualAttention Trainium2 kernel (Bass/Tile), 8-core data-parallel.

Contract: kernel(**inputs) takes the FULL unsharded inputs, shards batch b
across the 8 NeuronCores (one batch per core), and returns the full
(8, 128, 4096) float32 output.

Math (per batch b, per head h, D=128, X=4096):
  y_p   = dwconv3(x) + t_p/s_p           (p in q,k,v; BN folded so that
                                          W_eff_p @ y_p == pw_p @ BN(conv))
  k     = W_eff_k @ y_k ; sk = softmax(k over d)
  kat   = SCALE * q^T @ sk               (SCALE folded into W_q)
  gout  = GW @ q + gb ; sig = sigmoid(gout)
  out_h = v @ kat + sig^T * v
  out   = out_w @ merge(out_h) + out_b

Kernel factorizations (validated against the jax reference):
  * q is never materialized: kat_h = wtq_h^T @ R_h with
    R_h = y_q^T^T ... i.e. R[c,(h,d)] = sum_x y_q[c,x] sk'[x,(h,d)],
    where y_q INCLUDES the conv bias t'_q, so the rank-1 bias/sigma
    correction of the baseline is unnecessary.  y_qT is produced from
    y_q by DMA transpose (bf16), not by extra PE work.
  * v@kat through the output projection collapses to W3 @ y_v with
    W3 = sum_h outw_h @ (Wv_h^T @ kat_h)^T, computed on-chip from the
    tiny per-head kat matrices.
  * everything flows in bf16 (PSUM accumulation in fp32); final output
    is fp32.
"""
import numpy as np
import ml_dtypes

import concourse.bass as bass
import concourse.tile as tile
from concourse import bacc, mybir
from concourse.bass_utils import run_bass_kernel_spmd

F32 = mybir.dt.float32
BF16 = mybir.dt.bfloat16
AF = mybir.ActivationFunctionType
ALU = mybir.AluOpType

B = 8
DIM = 128
HEADS = 8
INNER = DIM * HEADS
X = 4096
EPS = 1e-5
SCALE = DIM ** -0.5
NT = X // 128          # 32 x-tiles of 128
NCH = X // 512         # 8 chunks of 512
NCB = X // 1024        # 4 chunks of 1024

_NC = None
TRACE = False
LAST_EXEC_NS = None


def _bf(a):
    return np.ascontiguousarray(np.asarray(a, np.float32).astype(ml_dtypes.bfloat16))


def _prep(inputs):
    """Host-side weight folding. Returns dict of DRAM input arrays."""
    f = lambda k: np.asarray(inputs[k], np.float32)
    wt = {}
    tprime = {}
    diag_cols = []
    for p in ("q", "k", "v"):
        s = f(p + "_g") / np.sqrt(f(p + "_v") + EPS)        # (128,)
        t = f(p + "_b") - f(p + "_m") * s
        tprime[p] = t / s
        w_eff = f(p + "_pw") * s[None, :]                    # (1024, 128)
        wt[p] = np.ascontiguousarray(w_eff.T)                # (128, 1024)
        dw = f(p + "_dw")[:, 0, :]                           # (128, 3)
        for j in range(3):
            diag_cols.append(np.diag(dw[:, j]).astype(np.float32))
    s_gt = f("gt_g") / np.sqrt(f("gt_v") + EPS)
    t_gt = f("gt_b") - f("gt_m") * s_gt
    gw = f("gt_pw") * (f("gt_dw")[:, 0, 0] * s_gt)[None, :]  # (128, 128)
    gb = f("gt_pw") @ t_gt                                   # (128,)
    w_eff_q = wt["q"].T                                      # (1024, 128)
    gqt = np.concatenate(
        [(gw @ w_eff_q[h * 128:(h + 1) * 128, :]).T for h in range(HEADS)], axis=1
    )                                                        # (128 i, 1024 h*o)
    out_w = f("out_w")                                       # (128, 1024)
    outwt = np.concatenate(
        [np.ascontiguousarray(out_w[:, h * 128:(h + 1) * 128].T) for h in range(HEADS)],
        axis=1,
    )                                                        # (128 d, 1024 h*o)
    wvdm = np.concatenate(
        [wt["v"].T[h * 128:(h + 1) * 128, :] for h in range(HEADS)], axis=1
    )                                                        # (128 d, 1024 h*i)
    diag = np.concatenate(diag_cols, axis=1)                 # (128, 1152)
    wtq_s = wt["q"] * SCALE                                  # (128 i, 1024 d)
    biasp = np.stack(
        [tprime["q"], tprime["k"], tprime["v"], gb, f("out_b")], axis=1
    )                                                        # (128, 5)
    return {
        "wtk": _bf(wt["k"]),
        "wtv": _bf(wt["v"]),
        "gqt": _bf(gqt),
        "outwt": _bf(outwt),
        "wvdm": _bf(wvdm),
        "diag": _bf(diag),
        "biasp": np.ascontiguousarray(biasp.astype(np.float32)),
        "wtqr": _bf(wtq_s),
    }


def _build():
    nc = bacc.Bacc("TRN2", target_bir_lowering=False, debug=False, num_devices=B)
    xb_d = nc.dram_tensor("xb", [128, X + 2], BF16, kind="ExternalInput").ap()
    wtk_d = nc.dram_tensor("wtk", [128, INNER], BF16, kind="ExternalInput").ap()
    wtv_d = nc.dram_tensor("wtv", [128, INNER], BF16, kind="ExternalInput").ap()
    gqt_d = nc.dram_tensor("gqt", [128, INNER], BF16, kind="ExternalInput").ap()
    outwt_d = nc.dram_tensor("outwt", [128, INNER], BF16, kind="ExternalInput").ap()
    wvdm_d = nc.dram_tensor("wvdm", [128, INNER], BF16, kind="ExternalInput").ap()
    diag_d = nc.dram_tensor("diag", [128, 9 * 128], BF16, kind="ExternalInput").ap()
    biasp_d = nc.dram_tensor("biasp", [128, 5], F32, kind="ExternalInput").ap()
    wtqr_d = nc.dram_tensor("wtqr", [128, INNER], BF16, kind="ExternalInput").ap()
    out_d = nc.dram_tensor("out", [128, X], F32, kind="ExternalOutput").ap()

    # host biasp column order: q, k, v, gb, out_b
    BQ, BK, BV, BG, BO = 0, 1, 2, 3, 4

    with tile.TileContext(nc) as tc:
        with (
            tc.tile_pool(name="const", bufs=1) as cp,
        ):
            wtk = cp.tile([128, INNER], BF16)
            wtv = cp.tile([128, INNER], BF16)
            gqt = cp.tile([128, INNER], BF16)
            outwt = cp.tile([128, INNER], BF16)
            wvdm = cp.tile([128, INNER], BF16)
            wtqr = cp.tile([128, INNER], BF16)
            diag = cp.tile([128, 9 * 128], BF16)
            biasp = cp.tile([128, 5], F32)
            yq = cp.tile([128, X], BF16, tag="yq")
            yk = cp.tile([128, X], BF16, tag="yk")
            yv = cp.tile([128, X], BF16, tag="yv")
            yqt = cp.tile([128, X], BF16, tag="yqt")
            sksb = cp.tile([128, NT * 1024], BF16, tag="sksb")
            zt = cp.tile([128, NT * 8], F32, tag="zt")
            zi = cp.tile([128, NT * 8], F32, tag="zi")
            zib = cp.tile([128, NT * 8], BF16, tag="zib")
            r_sb = cp.tile([128, INNER], BF16, tag="rsb")
            kat_sb = cp.tile([128, INNER], BF16, tag="katsb")
            m2_sb = cp.tile([128, INNER], BF16, tag="m2sb")
            w3t_sb = cp.tile([128, 128], BF16, tag="w3t")

            xb = cp.tile([128, X + 2], BF16, tag="xb")
            nc.sync.dma_start(out=xb, in_=xb_d)
            nc.sync.dma_start(out=diag, in_=diag_d)
            nc.sync.dma_start(out=biasp, in_=biasp_d)
            for sb_t, dr in ((wtk, wtk_d), (wtqr, wtqr_d), (wvdm, wvdm_d),
                             (outwt, outwt_d), (gqt, gqt_d), (wtv, wtv_d)):
                nc.sync.dma_start(out=sb_t, in_=dr)

            ys = {"q": yq, "k": yk, "v": yv}
            bcol = {"q": BQ, "k": BK, "v": BV}
            # diag block index per path (host order q,k,v x taps)
            dbase = {"q": 0, "k": 3, "v": 6}

            # ---- y-stage: depthwise conv via 3 shifted diagonal matmuls ----
            # k first so phase A can start earliest; q second (feeds the
            # DMA transposes for yqt), v last.
            with (
                tc.tile_pool(name="yps", bufs=2, space="PSUM") as yps,
            ):
                for p in ("k", "q", "v"):
                    for c in range(NCB):
                        pt = yps.tile([128, 1024], F32, tag="yps")
                        for j in range(3):
                            dsl = diag[:, (dbase[p] + j) * 128:(dbase[p] + j + 1) * 128]
                            for u in range(2):
                                nc.tensor.matmul(
                                    pt[:, u * 512:(u + 1) * 512], dsl,
                                    xb[:, c * 1024 + u * 512 + j:
                                       c * 1024 + u * 512 + j + 512],
                                    start=(j == 0), stop=(j == 2),
                                    skip_group_check=True,
                                )
                        nc.scalar.activation(
                            ys[p][:, c * 1024:(c + 1) * 1024], pt,
                            AF.Identity, bias=biasp[:, bcol[p]:bcol[p] + 1],
                        )
                        if p == "q":
                            for tt in range(8):
                                t = c * 8 + tt
                                nc.sync.dma_start_transpose(
                                    yqt[:, t * 128:(t + 1) * 128],
                                    yq[:, t * 128:(t + 1) * 128],
                                )

            # ---- phase A: K -> exp -> z -> normalize -> R -> kat -> W3 ----
            with (
                tc.tile_pool(name="kqps", bufs=2, space="PSUM") as kqps,
                tc.tile_pool(name="rps", bufs=1, space="PSUM") as rps,
            ):
                r_ps = rps.tile([128, 1024], F32, tag="r")
                for t in range(NT):
                    kt = kqps.tile([128, 1024], F32, tag="kq")
                    ykt = yk[:, t * 128:(t + 1) * 128]
                    nc.tensor.matmul(kt[:, 0:512], ykt, wtk[:, 0:512],
                                     start=True, stop=True)
                    nc.tensor.matmul(kt[:, 512:1024], ykt, wtk[:, 512:1024],
                                     start=True, stop=True)
                    nc.scalar.activation(
                        sksb[:, t * 1024:(t + 1) * 1024], kt, AF.Exp,
                    )
                    # z: per-head row sums of exp(k) on DVE
                    nc.vector.tensor_reduce(
                        zt[:, t * 8:(t + 1) * 8],
                        sksb[:, t * 1024:(t + 1) * 1024].rearrange(
                            "p (h d) -> p h d", h=8
                        ),
                        mybir.AxisListType.X, ALU.add,
                    )
                    if t % 4 == 3:
                        t0 = t - 3
                        nc.vector.reciprocal(
                            zi[:, t0 * 8:(t + 1) * 8], zt[:, t0 * 8:(t + 1) * 8]
                        )
                        nc.vector.tensor_copy(
                            zib[:, t0 * 8:(t + 1) * 8], zi[:, t0 * 8:(t + 1) * 8]
                        )
                for t in range(NT):
                    # normalize sk in place (gpsimd; broadcast per head)
                    skv = sksb[:, t * 1024:(t + 1) * 1024].rearrange(
                        "p (h d) -> p h d", h=8
                    )
                    zb = zib[:, t * 8:(t + 1) * 8][:, :, None].to_broadcast(
                        (128, 8, 128)
                    )
                    nc.gpsimd.tensor_tensor(skv, skv, zb, ALU.mult)
                    # R accumulation over x tiles
                    yqtt = yqt[:, t * 128:(t + 1) * 128]
                    nc.tensor.matmul(
                        r_ps[:, 0:512], yqtt, sksb[:, t * 1024:t * 1024 + 512],
                        start=(t == 0), stop=(t == NT - 1), skip_group_check=True,
                    )
                    nc.tensor.matmul(
                        r_ps[:, 512:1024], yqtt,
                        sksb[:, t * 1024 + 512:(t + 1) * 1024],
                        start=(t == 0), stop=(t == NT - 1), skip_group_check=True,
                    )
                nc.vector.tensor_copy(r_sb, r_ps)

            # kat -> M2 -> W3T per head (tiny matmul chain)
            with (
                tc.tile_pool(name="smps", bufs=2, space="PSUM") as smps,
            ):
                w3t_ps = smps.tile([128, 128], F32, tag="w3tp")
                for h in range(HEADS):
                    hsl = slice(h * 128, (h + 1) * 128)
                    kat_ps = smps.tile([128, 128], F32, tag="katp")
                    nc.tensor.matmul(
                        kat_ps, wtqr[:, hsl], r_sb[:, hsl],
                        start=True, stop=True, skip_group_check=True,
                    )
                    nc.vector.tensor_copy(kat_sb[:, hsl], kat_ps)
                    m2_ps = smps.tile([128, 128], F32, tag="m2p")
                    nc.tensor.matmul(
                        m2_ps, kat_sb[:, hsl], wvdm[:, hsl],
                        start=True, stop=True, skip_group_check=True,
                    )
                    nc.vector.tensor_copy(m2_sb[:, hsl], m2_ps)
                    nc.tensor.matmul(
                        w3t_ps, m2_sb[:, hsl], outwt[:, hsl],
                        start=(h == 0), stop=(h == HEADS - 1),
                        skip_group_check=True,
                    )
                nc.vector.tensor_copy(w3t_sb, w3t_ps)

            # ---- phase B: gate + final projection per 512-chunk ----
            with (
                tc.tile_pool(name="goutps", bufs=2, space="PSUM") as goutps,
                tc.tile_pool(name="vps", bufs=2, space="PSUM") as vps,
                tc.tile_pool(name="finps", bufs=2, space="PSUM") as finps,
                tc.tile_pool(name="bpool", bufs=3) as bp,
            ):
                for c in range(NCH):
                    csl = slice(c * 512, (c + 1) * 512)
                    fin_ps = finps.tile([128, 512], F32, tag="fin")
                    for hp in range(HEADS // 2):
                        g_ps = goutps.tile([128, 1024], F32, tag="gout")
                        for d in range(2):
                            h = hp * 2 + d
                            nc.tensor.matmul(
                                g_ps[:, d * 512:(d + 1) * 512],
                                gqt[:, h * 128:(h + 1) * 128],
                                yq[:, csl], start=True, stop=True,
                            )
                        sig = bp.tile([128, 1024], BF16, tag="sig")
                        nc.scalar.activation(
                            sig, g_ps, AF.Sigmoid, bias=biasp[:, BG:BG + 1],
                        )
                        for d in range(2):
                            h = hp * 2 + d
                            v_ps = vps.tile([128, 512], F32, tag="vp")
                            nc.tensor.matmul(
                                v_ps, wtv[:, h * 128:(h + 1) * 128],
                                yv[:, csl], start=True, stop=True,
                            )
                            gate = bp.tile([128, 512], BF16, tag="gate")
                            nc.vector.tensor_tensor(
                                gate, v_ps, sig[:, d * 512:(d + 1) * 512], ALU.mult
                            )
                            nc.tensor.matmul(
                                fin_ps, outwt[:, h * 128:(h + 1) * 128], gate,
                                start=(h == 0), stop=False, skip_group_check=True,
                            )
                    nc.tensor.matmul(
                        fin_ps, w3t_sb, yv[:, csl],
                        start=False, stop=True, skip_group_check=True,
                    )
                    fin_sb = bp.tile([128, 512], F32, tag="finsb")
                    nc.scalar.activation(
                        fin_sb, fin_ps, AF.Identity, bias=biasp[:, BO:BO + 1]
                    )
                    nc.sync.dma_start(out=out_d[:, csl], in_=fin_sb)

    nc.compile()
    return nc


def kernel(**inputs):
    global _NC, LAST_EXEC_NS
    host = _prep(inputs)
    if _NC is None:
        _NC = _build()
    x = np.asarray(inputs["x"], np.float32)
    in_maps = []
    for b in range(B):
        xp = np.pad(x[b], ((0, 0), (1, 1)))
        m = {"xb": _bf(xp)}
        m.update(host)
        in_maps.append(m)
    res = run_bass_kernel_spmd(
        _NC, in_maps, core_ids=list(range(B)), trace=TRACE
    )
    LAST_EXEC_NS = res.exec_time_ns
    return np.stack([r["out"] for r in res.results]).astype(np.float32)
